# revision 1
# baseline (speedup 1.0000x reference)
"""Trainium2 Bass kernel for nn_Node_Transformation.

Computes, for row n:
    out[n] = emb_weight[node_type[n]]                 if node_type[n] != item_id
             x[n] @ W.T + b                           if node_type[n] == item_id

Equivalent formulation used on device (exact, float-add commutative):
    table2       = emb_weight with row item_id replaced by b
    out[n]       = table2[node_type[n]] + mask[n] * (x[n] @ W.T)

Sharding: data-parallel over N across 8 NeuronCores. Weights/table replicated.
Per-core rows are laid out "partition-major": global (in-shard) row index
r = p*F + f  for partition p in [0,128) and tile column f in [0,F).
"""

import os
import numpy as np

import concourse.bass as bass
import concourse.bacc as bacc
import concourse.mybir as mybir
from concourse.tile import TileContext
from concourse.bass import IndirectOffsetOnAxis
from concourse.bass_utils import run_bass_kernel_spmd
from concourse.masks import make_identity

# ---- problem constants (hardcoded per contest contract) ----
N = 500000
IN_CH = 256
HID = 128
NUM_T = 8
NCORES = 8
P = 128
NSH = N // NCORES          # 62500 real rows per core
F = (NSH + P - 1) // P     # 489 tile columns
PAD = P * F                # 62592 padded rows per core

_CACHE = {}


def _ensure_axon_profile_hook():
    """bass_utils' trace path imports antenv.axon_hooks, which this image
    lacks. Register an equivalent module backed by the axon PJRT .so so
    trace=True (or BASS_TRACE=1) works instead of crashing."""
    try:
        import antenv.axon_hooks  # noqa: F401
        return
    except ImportError:
        pass
    import sys
    import types

    hook = None
    try:
        from trn_agent_boot.trn_boot import _ntff_profile_via_ctypes

        hook = _ntff_profile_via_ctypes("/opt/axon/libaxon_pjrt.so")
    except Exception:
        hook = None
    mod = types.ModuleType("antenv.axon_hooks")
    mod.get_axon_ntff_profile_hook = lambda: hook
    mod.set_axon_ntff_profile_hook = lambda h: None
    sys.modules["antenv.axon_hooks"] = mod
    try:
        import antenv

        antenv.axon_hooks = mod
    except ImportError:
        pass


def _build(item: int) -> bass.Bass:
    nc = bacc.Bacc("TRN2")
    f32 = mybir.dt.float32
    i32 = mybir.dt.int32

    x_d = nc.dram_tensor("x", [PAD, IN_CH], f32, kind="ExternalInput")
    nt_d = nc.dram_tensor("nt", [PAD], i32, kind="ExternalInput")
    t2_d = nc.dram_tensor("table2", [NUM_T, HID], f32, kind="ExternalInput")
    wt_d = nc.dram_tensor("wt", [IN_CH, HID], f32, kind="ExternalInput")
    out_d = nc.dram_tensor("out", [PAD, HID], f32, kind="ExternalOutput")

    x_v = x_d[:].rearrange("(p f) c -> p f c", p=P)     # [128, F, 256]
    nt_v = nt_d[:].rearrange("(p f) -> p f", p=P)       # [128, F]
    out_v = out_d[:].rearrange("(p f) h -> p f h", p=P) # [128, F, 128]

    with TileContext(nc) as tc:
        with (
            tc.tile_pool(name="singles", bufs=1) as singles,
            tc.tile_pool(name="xp", bufs=4) as xpool,
            tc.tile_pool(name="tp", bufs=4) as tpool,
            tc.tile_pool(name="op", bufs=4) as opool,
            tc.tile_pool(name="ps", bufs=2, space="PSUM") as pspool,
        ):
            ident = singles.tile([P, P], f32)
            make_identity(nc, ident)

            wt_s = singles.tile([P, 2, HID], f32)
            nc.sync.dma_start(out=wt_s[:], in_=wt_d[:].rearrange("(c k) h -> k c h", c=2))

            nt_all = singles.tile([P, F], i32)
            nc.sync.dma_start(out=nt_all[:], in_=nt_v)
            ntf = singles.tile([P, F], f32)
            nc.vector.tensor_copy(ntf[:], nt_all[:])
            eq_all = singles.tile([P, F], f32)
            nc.vector.tensor_scalar(
                out=eq_all[:], in0=ntf[:], scalar1=float(item), scalar2=None,
                op0=mybir.AluOpType.is_equal,
            )

            for f in range(F):
                x_t = xpool.tile([P, IN_CH], f32, tag="x")
                nc.sync.dma_start(out=x_t[:], in_=x_v[:, f, :])
                xm = xpool.tile([P, IN_CH], f32, tag="xm")
                nc.vector.tensor_tensor(
                    out=xm[:], in0=x_t[:],
                    in1=eq_all[:, f : f + 1].to_broadcast([P, IN_CH]),
                    op=mybir.AluOpType.mult,
                )

                pt = pspool.tile([P, P], f32, tag="pt")
                pt2 = pspool.tile([P, P], f32, tag="pt2")
                nc.tensor.transpose(pt[:], xm[:, 0:P], ident[:])
                nc.tensor.transpose(pt2[:], xm[:, P : 2 * P], ident[:])
                xt = tpool.tile([P, 2, P], f32, tag="xt")
                nc.vector.tensor_copy(xt[:, 0, :], pt[:])
                nc.vector.tensor_copy(xt[:, 1, :], pt2[:])

                lin = pspool.tile([P, HID], f32, tag="lin")
                nc.tensor.matmul(out=lin[:], lhsT=xt[:, 0, :], rhs=wt_s[:, 0, :],
                                 start=True, stop=False)
                nc.tensor.matmul(out=lin[:], lhsT=xt[:, 1, :], rhs=wt_s[:, 1, :],
                                 start=False, stop=True)

                emb_t = opool.tile([P, HID], f32, tag="emb")
                nc.gpsimd.indirect_dma_start(
                    out=emb_t[:], out_offset=None, in_=t2_d[:],
                    in_offset=IndirectOffsetOnAxis(ap=nt_all[:, f : f + 1], axis=0),
                )
                o_t = opool.tile([P, HID], f32, tag="o")
                nc.vector.tensor_tensor(out=o_t[:], in0=emb_t[:], in1=lin[:],
                                        op=mybir.AluOpType.add)
                nc.scalar.dma_start(out=out_v[:, f, :], in_=o_t[:])
    nc.compile()
    return nc


def _prepare(inputs):
    x = np.asarray(inputs["x"], dtype=np.float32)
    nt = np.asarray(inputs["node_type"]).astype(np.int32)
    item = int(np.asarray(inputs["item_id"]))
    emb = np.asarray(inputs["emb_weight"], dtype=np.float32)
    W = np.asarray(inputs["W"], dtype=np.float32)
    b = np.asarray(inputs["b"], dtype=np.float32)

    table2 = emb.copy()
    table2[item] = b
    wt = np.ascontiguousarray(W.T)  # [IN_CH, HID]
    pad_val = np.int32((item + 1) % NUM_T)  # never selected

    in_maps = []
    for c in range(NCORES):
        xp = np.zeros((PAD, IN_CH), np.float32)
        xp[:NSH] = x[c * NSH : (c + 1) * NSH]
        ntp = np.full(PAD, pad_val, np.int32)
        ntp[:NSH] = nt[c * NSH : (c + 1) * NSH]
        in_maps.append({"x": xp, "nt": ntp, "table2": table2, "wt": wt})
    return item, in_maps


def _run(inputs, trace=False):
    _ensure_axon_profile_hook()
    item, in_maps = _prepare(inputs)
    if item not in _CACHE:
        _CACHE[item] = _build(item)
    nc = _CACHE[item]
    res = run_bass_kernel_spmd(nc, in_maps, core_ids=list(range(NCORES)), trace=trace)
    out = np.empty((N, HID), np.float32)
    for c in range(NCORES):
        out[c * NSH : (c + 1) * NSH] = res.results[c]["out"][:NSH]
    return out, res


def kernel(**inputs) -> np.ndarray:
    out, _ = _run(inputs, trace=bool(os.environ.get("KERNEL_TRACE")))
    return out



# revision 3
# speedup vs baseline: 5.5379x; 5.5379x over previous
"""Trainium2 Bass kernel for nn_Node_Transformation.

Reference semantics, for row n:
    out[n] = x[n] @ W.T + b            if node_type[n] == item_id
             emb_weight[node_type[n]]  otherwise

Only ~1/8 of rows take the linear path, so the kernel is split:

  Dense part (all rows): out_dense[n] = emb_weight[node_type[n]], computed as a
  one-hot matmul: outT[h, r] = sum_t table[t, h] * onehot[t, r], with the tiny
  table as the stationary operand (loaded once) and the host-built one-hot
  indicator streaming as rhs. Output is produced hid-major ("outT") so the
  per-group varying operand is the streaming one (no LDWEIGHTS churn).

  Sparse part (selected rows only): row indices where node_type == item_id are
  computed on host (metadata only); the kernel gathers just those x rows via
  indirect DMA (1/8 of x traffic), transposes them on the PE, and computes
  lin = x_sel @ W.T + b into a compact second output. The host scatters those
  rows over the dense result while unsharding.

Everything on-device is bf16 (psum accumulation in f32); the correctness gate
is a scale-relative 2e-2 absmax, bf16 error is ~4e-3.

Sharding: data-parallel over N across 8 NeuronCores; weights/table replicated.
"""

import os
import numpy as np
import ml_dtypes

import concourse.bass as bass
import concourse.bacc as bacc
import concourse.mybir as mybir
from concourse.tile import TileContext
from concourse.bass import IndirectOffsetOnAxis
from concourse.bass_utils import run_bass_kernel_spmd
from concourse.masks import make_identity

# ---- problem constants (hardcoded per contest contract) ----
N = 500000
IN_CH = 256
HID = 128
NUM_T = 8
NCORES = 8
NSH = N // NCORES          # 62500 rows per core
GRP = 512                  # rows per matmul group (one PSUM bank, bf16 rhs)
NG = (NSH + GRP - 1) // GRP            # 123 dense groups
PADR = NG * GRP                        # 62976 padded rows per core
OHC = 8192                 # one-hot columns loaded per DMA (16 groups)
SLABG = 8                  # dense groups per output slab (4096 cols per DMA)

BF16 = ml_dtypes.bfloat16

_CACHE = {}


def _ensure_axon_profile_hook():
    """bass_utils' trace path imports antenv.axon_hooks, which this image
    lacks. Register an equivalent module backed by the axon PJRT .so so
    trace=True (or BASS_TRACE=1) works instead of crashing."""
    try:
        import antenv.axon_hooks  # noqa: F401
        return
    except ImportError:
        pass
    import sys
    import types

    hook = None
    try:
        from trn_agent_boot.trn_boot import _ntff_profile_via_ctypes

        hook = _ntff_profile_via_ctypes("/opt/axon/libaxon_pjrt.so")
    except Exception:
        hook = None
    mod = types.ModuleType("antenv.axon_hooks")
    mod.get_axon_ntff_profile_hook = lambda: hook
    mod.set_axon_ntff_profile_hook = lambda h: None
    sys.modules["antenv.axon_hooks"] = mod
    try:
        import antenv

        antenv.axon_hooks = mod
    except ImportError:
        pass


def _build(tsel: int) -> bass.Bass:
    nc = bacc.Bacc("TRN2")
    f32 = mybir.dt.float32
    bf16 = mybir.dt.bfloat16
    i32 = mybir.dt.int32

    x_d = nc.dram_tensor("x", [NSH, IN_CH], bf16, kind="ExternalInput")
    oh_d = nc.dram_tensor("oh", [NUM_T, PADR], bf16, kind="ExternalInput")
    idx_d = nc.dram_tensor("idx", [128, tsel], i32, kind="ExternalInput")
    t2_d = nc.dram_tensor("t2", [NUM_T, HID], bf16, kind="ExternalInput")
    wt_d = nc.dram_tensor("wt", [IN_CH, HID], bf16, kind="ExternalInput")
    bb_d = nc.dram_tensor("bb", [HID, 1], f32, kind="ExternalInput")
    outT_d = nc.dram_tensor("outT", [HID, PADR], bf16, kind="ExternalOutput")
    o2T_d = nc.dram_tensor("o2T", [HID, tsel * 128], bf16, kind="ExternalOutput")

    sgroups = tsel // 4                 # sel tiles are processed 4 per group
    # spread the sel groups evenly through the dense loop
    sel_at = set(int(round((i + 0.5) * NG / sgroups)) for i in range(sgroups))
    assert len(sel_at) == sgroups

    with TileContext(nc) as tc:
        with (
            tc.tile_pool(name="singles", bufs=1) as singles,
            tc.tile_pool(name="ohp", bufs=2) as ohpool,
            tc.tile_pool(name="osl", bufs=3) as opool,
            tc.tile_pool(name="xsp", bufs=6) as xpool,
            tc.tile_pool(name="xtp", bufs=2) as xtpool,
            tc.tile_pool(name="o2p", bufs=2) as o2pool,
            tc.tile_pool(name="psd", bufs=3, space="PSUM") as psd,
            tc.tile_pool(name="pst", bufs=3, space="PSUM") as pst,
            tc.tile_pool(name="psl", bufs=2, space="PSUM") as psl,
        ):
            ident = singles.tile([128, 128], bf16)
            make_identity(nc, ident)

            t2_s = singles.tile([NUM_T, HID], bf16)
            nc.sync.dma_start(out=t2_s[:], in_=t2_d[:])
            wt_s = singles.tile([128, 2, HID], bf16)
            nc.sync.dma_start(out=wt_s[:], in_=wt_d[:].rearrange("(k c) h -> c k h", k=2))
            bb_s = singles.tile([HID, 1], f32)
            nc.sync.dma_start(out=bb_s[:], in_=bb_d[:])
            idx_s = singles.tile([128, tsel], i32)
            nc.sync.dma_start(out=idx_s[:], in_=idx_d[:])

            oh_tile = None
            oslab = None
            slab_g0 = 0
            sel_emitted = 0

            def emit_sel_group(sg):
                xsT = xtpool.tile([128, 2, GRP], bf16, tag="xsT")
                for j in range(4):
                    t = sg * 4 + j
                    xs = xpool.tile([128, IN_CH], bf16, tag="xs")
                    nc.gpsimd.indirect_dma_start(
                        out=xs[:], out_offset=None, in_=x_d[:],
                        in_offset=IndirectOffsetOnAxis(ap=idx_s[:, t : t + 1], axis=0),
                    )
                    pt = pst.tile([128, IN_CH], bf16, tag="pt")
                    nc.tensor.transpose(pt[:, 0:128], xs[:, 0:128], ident[:])
                    nc.tensor.transpose(pt[:, 128:256], xs[:, 128:256], ident[:])
                    nc.vector.tensor_copy(xsT[:, 0, j * 128 : (j + 1) * 128], pt[:, 0:128])
                    nc.vector.tensor_copy(xsT[:, 1, j * 128 : (j + 1) * 128], pt[:, 128:256])
                lp = psl.tile([HID, GRP], f32, tag="lp")
                nc.tensor.matmul(out=lp[:], lhsT=wt_s[:, 0, :], rhs=xsT[:, 0, :],
                                 start=True, stop=False)
                nc.tensor.matmul(out=lp[:], lhsT=wt_s[:, 1, :], rhs=xsT[:, 1, :],
                                 start=False, stop=True)
                o2 = o2pool.tile([HID, GRP], bf16, tag="o2")
                nc.scalar.activation(out=o2[:], in_=lp[:],
                                     func=mybir.ActivationFunctionType.Identity,
                                     bias=bb_s[:, 0:1], scale=1.0)
                nc.scalar.dma_start(out=o2T_d[:, sg * GRP : (sg + 1) * GRP], in_=o2[:])

            for g in range(NG):
                c0 = g * GRP
                if g % (OHC // GRP) == 0:
                    oh_tile = ohpool.tile([NUM_T, OHC], bf16, tag="oh")
                    lo = g * GRP
                    hi = min(lo + OHC, PADR)
                    nc.sync.dma_start(out=oh_tile[:, 0 : hi - lo], in_=oh_d[:, lo:hi])
                    oh_base = lo
                if g % SLABG == 0:
                    oslab = opool.tile([HID, SLABG * GRP], bf16, tag="oslab")
                    slab_g0 = g

                pd = psd.tile([HID, GRP], f32, tag="pd")
                nc.tensor.matmul(
                    out=pd[:], lhsT=t2_s[:],
                    rhs=oh_tile[:, c0 - oh_base : c0 - oh_base + GRP],
                    start=True, stop=True,
                )
                so = (g - slab_g0) * GRP
                nc.vector.tensor_copy(oslab[:, so : so + GRP], pd[:])

                if g == slab_g0 + SLABG - 1 or g == NG - 1:
                    lo = slab_g0 * GRP
                    hi = (g + 1) * GRP
                    nc.scalar.dma_start(out=outT_d[:, lo:hi], in_=oslab[:, 0 : hi - lo])

                if g in sel_at and sel_emitted < sgroups:
                    emit_sel_group(sel_emitted)
                    sel_emitted += 1

            while sel_emitted < sgroups:
                emit_sel_group(sel_emitted)
                sel_emitted += 1

    nc.compile()
    return nc


def _prepare(inputs):
    x = np.asarray(inputs["x"])
    nt = np.asarray(inputs["node_type"]).astype(np.int64)
    item = int(np.asarray(inputs["item_id"]))
    emb = np.asarray(inputs["emb_weight"], dtype=np.float32)
    W = np.asarray(inputs["W"], dtype=np.float32)
    b = np.asarray(inputs["b"], dtype=np.float32)

    t2 = emb.astype(BF16)
    wt = np.ascontiguousarray(W.T).astype(BF16)
    bb = b.astype(np.float32).reshape(HID, 1)

    sels = []
    max_nsel = 0
    for c in range(NCORES):
        sel = np.flatnonzero(nt[c * NSH : (c + 1) * NSH] == item).astype(np.int32)
        sels.append(sel)
        max_nsel = max(max_nsel, len(sel))
    tsel = max(64, ((-(-max_nsel // 128) + 3) // 4 + 1) * 4)

    in_maps = []
    for c in range(NCORES):
        nt_sh = nt[c * NSH : (c + 1) * NSH]
        xb = np.ascontiguousarray(x[c * NSH : (c + 1) * NSH]).astype(BF16)

        oh = np.zeros((NUM_T, PADR), dtype=BF16)
        for t in range(NUM_T):
            oh[t, :NSH] = (nt_sh == t)

        idxp = np.zeros(tsel * 128, dtype=np.int32)
        idxp[: len(sels[c])] = sels[c]
        idx_pm = np.ascontiguousarray(idxp.reshape(tsel, 128).T)

        in_maps.append({"x": xb, "oh": oh, "idx": idx_pm,
                        "t2": t2, "wt": wt, "bb": bb})
    return tsel, sels, in_maps


def _run(inputs, trace=False):
    _ensure_axon_profile_hook()
    tsel, sels, in_maps = _prepare(inputs)
    if tsel not in _CACHE:
        _CACHE[tsel] = _build(tsel)
    nc = _CACHE[tsel]
    res = run_bass_kernel_spmd(nc, in_maps, core_ids=list(range(NCORES)), trace=trace)
    out = np.empty((N, HID), np.float32)
    for c in range(NCORES):
        outT = res.results[c]["outT"]          # [HID, PADR] bf16
        osh = out[c * NSH : (c + 1) * NSH]
        osh[:] = outT[:, :NSH].astype(np.float32).T
        sel = sels[c]
        if len(sel):
            o2T = res.results[c]["o2T"]        # [HID, tsel*128] bf16
            osh[sel] = o2T[:, : len(sel)].astype(np.float32).T
    return out, res


def kernel(**inputs) -> np.ndarray:
    out, _ = _run(inputs, trace=bool(os.environ.get("KERNEL_TRACE")))
    return out
